# revision 2
# baseline (speedup 1.0000x reference)
"""Chamfer-distance (CDLoss) kernel for 8x Trainium2 NeuronCores.

Strategy (data-parallel, per spec sharding hint):
  - 8 cores = 4 batches x 2 halves. Core c handles batch b=c//2, query-half
    h=c%2 for BOTH directions of the chamfer distance:
      dir A: queries = x[b, h*4096:(h+1)*4096], candidates = y[b] (all 8192)
      dir B: queries = y[b, h*4096:(h+1)*4096], candidates = x[b]
  - On device, per query-tile of 128 (partition dim), the full distance
    matrix D[i,j] = |q_i|^2 + |c_j|^2 - 2 q_i.c_j is produced by TensorE as
    an augmented matmul (K=13) in float32r (tf32) with an exact hi/lo split
    compensation, 512 candidates per PSUM bank.
  - Min-reduction consumes PSUM via two paths in parallel:
      * ScalarE copies half the banks to SBUF (fp32),
      * VectorE runs a custom fused DVE op  min(Src0,Src1) + min-accumulate
        that eats one PSUM stream + one SBUF stream per cycle.
  - Per-query minima are summed on device to [128,1] per core; the final
    scalar is assembled on host: loss = (sum of all mins) * 0.5 / B.
"""

import re
import sys

sys.path.insert(0, "/opt/trn_rl_repo")

import numpy as np

import concourse.bacc as bacc
import concourse.mybir as mybir
import concourse.tile as tile
import concourse.dve_ops as dve_ops
from concourse.bass_interp import get_hw_module
from concourse.bass_utils import run_bass_kernel_spmd
from concourse.dve_ops import DveOp
from concourse.dve_spec import C0, Spec, Src0, Src1, minn

B, N, DIM = 4, 8192, 3
N_CORES = 8
HALF = N // 2              # queries per core per direction
QT = 128                   # queries per tile (partition dim)
NQT = HALF // QT           # query tiles per direction (32)
CT = 512                   # candidates per matmul (one PSUM bank)
SPAN = 1024                # candidates per PSUM span (2 banks)
K = 13                     # augmented contraction dim
F32 = mybir.dt.float32
F32R = mybir.dt.float32r

# DRAM input layout per core: one [K, 24576] fp32 tensor with columns
#   [rhs_dirA (8192) | rhs_dirB (8192) | lhs_dirA (4096) | lhs_dirB (4096)]
IN_COLS = 2 * N + 2 * HALF


# --- custom DVE op: out = min(in0, in1); accum_out = min(s0, min_k out) ----
def _min2_ref(in0, in1, s0, s1, imm2):
    b = np.minimum(in0, in1).astype(np.float32)
    m = b.reshape(b.shape[0], -1).min(axis=-1, keepdims=True)
    s0 = np.broadcast_to(np.asarray(s0, np.float32), m.shape)
    return b, np.minimum(s0, m).astype(np.float32)


def _register_min2():
    for op in dve_ops.OPS:
        if op.name == "MIN2_ACC_CD":
            return op
    op = DveOp(
        "MIN2_ACC_CD",
        Spec(body=minn(Src0, Src1), accum=minn, accum_init=C0, reference=_min2_ref),
        subdim=False,
        uops_sha={},
    )
    dve_ops.OPS.append(op)
    dve_ops.CUSTOM_DVE_SPECS[op.name] = op.spec
    dve_ops._SUB_OPCODE_FOR_NAME[op.name] = (
        dve_ops._CUSTOM_DVE_ROW_BASE + len(dve_ops.OPS) - 1
    )
    for ver in ("v3", "v4"):
        try:
            op.compile(ver)
        except ValueError as e:
            m = re.search(r'"([0-9a-f]{16})"', str(e))
            op.uops_sha[ver] = m.group(1)
            op.compile(ver)
    return op


MIN2 = _register_min2()


# --- device program ---------------------------------------------------------
def _build_program():
    nc = bacc.Bacc(
        trn_type="TRN2", debug=False, num_devices=N_CORES, enable_asserts=False
    )
    inp = nc.dram_tensor("inp", [K, IN_COLS], F32R, kind="ExternalInput")
    out = nc.dram_tensor("out", [128, 1], F32, kind="ExternalOutput")

    with tile.TileContext(nc) as tc:
        with (
            tc.tile_pool(name="const", bufs=1) as cpool,
            tc.tile_pool(name="psA", bufs=2, space="PSUM") as psA,
            tc.tile_pool(name="psB", bufs=2, space="PSUM") as psB,
            tc.tile_pool(name="stage", bufs=3) as stpool,
            tc.tile_pool(name="scr", bufs=2) as scrpool,
            tc.tile_pool(name="acc", bufs=4) as accpool,
        ):
            data = cpool.tile([K, IN_COLS], F32R)
            nc.sync.dma_start(out=data[:], in_=inp.ap())
            minbuf = cpool.tile([128, 2 * NQT], F32)
            sums = cpool.tile([128, 1], F32)

            data_r = data
            for d in range(2):
                rhs = data_r[:, d * N : (d + 1) * N]
                lhs = data_r[:, 2 * N + d * HALF : 2 * N + (d + 1) * HALF]
                for qt in range(NQT):
                    w = lhs[:, qt * QT : (qt + 1) * QT]
                    acc = accpool.tile([128, 4], F32)
                    for r in range(4):
                        base = r * 2 * SPAN
                        pa = psA.tile([128, SPAN], F32)
                        pb = psB.tile([128, SPAN], F32)
                        for half in range(2):
                            nc.tensor.matmul(
                                out=pa[:, half * CT : (half + 1) * CT],
                                lhsT=w,
                                rhs=rhs[:, base + half * CT : base + (half + 1) * CT],
                                start=True,
                                stop=True,
                            )
                        for half in range(2):
                            nc.tensor.matmul(
                                out=pb[:, half * CT : (half + 1) * CT],
                                lhsT=w,
                                rhs=rhs[
                                    :,
                                    base + SPAN + half * CT : base
                                    + SPAN
                                    + (half + 1) * CT,
                                ],
                                start=True,
                                stop=True,
                            )
                        sb = stpool.tile([128, SPAN], F32)
                        nc.scalar.copy(out=sb[:], in_=pb[:])
                        scr = scrpool.tile([128, SPAN], F32)
                        nc.vector._custom_dve(
                            MIN2,
                            out=scr[:],
                            in0=pa[:],
                            in1=sb[:],
                            s0=1.0e30,
                            accum_out=acc[:, r : r + 1],
                        )
                    col = d * NQT + qt
                    nc.vector.tensor_reduce(
                        out=minbuf[:, col : col + 1],
                        in_=acc[:],
                        axis=mybir.AxisListType.X,
                        op=mybir.AluOpType.min,
                    )
            nc.vector.tensor_reduce(
                out=sums[:],
                in_=minbuf[:],
                axis=mybir.AxisListType.X,
                op=mybir.AluOpType.add,
            )
            nc.sync.dma_start(out=out.ap(), in_=sums[:])

    nc.compile()
    nc.m = get_hw_module(nc.m)
    return nc


_NC = None


def _get_nc():
    global _NC
    if _NC is None:
        _NC = _build_program()
    return _NC


# --- host-side packing ------------------------------------------------------
def _tf32_split(a):
    """Round-to-nearest split of fp32 array into tf32 hi + fp32 lo."""
    a = np.ascontiguousarray(a, np.float32)
    u = a.view(np.uint32)
    hi = ((u + 0x1000) & 0xFFFFE000).view(np.float32)
    return hi, (a - hi).astype(np.float32)


def _pack_side(q, c):
    """Build (lhs [K, nq], rhs [K, nc]) for queries q [nq,3], candidates c [nc,3]."""
    nq, ncand = q.shape[0], c.shape[0]
    qq = (q.astype(np.float64) ** 2).sum(-1)
    cc = (c.astype(np.float64) ** 2).sum(-1)
    qqh, _ = _tf32_split(qq.astype(np.float32))
    qql = (qq - qqh.astype(np.float64)).astype(np.float32)
    cch, _ = _tf32_split(cc.astype(np.float32))
    ccl = (cc - cch.astype(np.float64)).astype(np.float32)
    qh, ql = _tf32_split(q)
    ch, cl = _tf32_split(c)

    lhs = np.empty((K, nq), np.float32)
    lhs[0] = qqh
    lhs[1] = qql
    lhs[2] = 1.0
    lhs[3] = 1.0
    lhs[4:7] = -2.0 * qh.T
    lhs[7:10] = -2.0 * qh.T
    lhs[10:13] = -2.0 * ql.T

    rhs = np.empty((K, ncand), np.float32)
    rhs[0] = 1.0
    rhs[1] = 1.0
    rhs[2] = cch
    rhs[3] = ccl
    rhs[4:7] = ch.T
    rhs[7:10] = cl.T
    rhs[10:13] = ch.T
    return lhs, rhs


def _pack_core(x, y, core):
    b, h = core // 2, core % 2
    sl = slice(h * HALF, (h + 1) * HALF)
    lhsA, rhsA = _pack_side(x[b, sl], y[b])
    lhsB, rhsB = _pack_side(y[b, sl], x[b])
    buf = np.empty((K, IN_COLS), np.float32)
    buf[:, 0:N] = rhsA
    buf[:, N : 2 * N] = rhsB
    buf[:, 2 * N : 2 * N + HALF] = lhsA
    buf[:, 2 * N + HALF :] = lhsB
    return buf


def kernel(gen_points_batch, train_points_dense_batch, _profile=None):
    x = np.ascontiguousarray(gen_points_batch, np.float32)
    y = np.ascontiguousarray(train_points_dense_batch, np.float32)
    assert x.shape == (B, N, DIM) and y.shape == (B, N, DIM)

    in_maps = [{"inp": _pack_core(x, y, c)} for c in range(N_CORES)]
    nc = _get_nc()
    res = run_bass_kernel_spmd(
        nc, in_maps, list(range(N_CORES)), **(_profile or {})
    )
    total = sum(
        res.results[c]["out"].astype(np.float64).sum() for c in range(N_CORES)
    )
    loss = np.float32(total * 0.5 / B)
    if _profile:
        kernel._last_result = res
    return loss


# revision 3
# speedup vs baseline: 1.0153x; 1.0153x over previous
"""Chamfer-distance (CDLoss) kernel for 8x Trainium2 NeuronCores.

Strategy (data-parallel, per spec sharding hint):
  - 8 cores = 4 batches x 2 halves. Core c handles batch b=c//2, query-half
    h=c%2 for BOTH directions of the chamfer distance:
      dir A: queries = x[b, h*4096:(h+1)*4096], candidates = y[b] (all 8192)
      dir B: queries = y[b, h*4096:(h+1)*4096], candidates = x[b]
  - On device, per query-tile of 128 (partition dim), the full distance
    matrix D[i,j] = |q_i|^2 + |c_j|^2 - 2 q_i.c_j is produced by TensorE as
    an augmented matmul (K=13) in float32r (tf32) with an exact hi/lo split
    compensation, 512 candidates per PSUM bank.
  - Min-reduction consumes PSUM via two paths in parallel:
      * ScalarE copies half the banks to SBUF (fp32),
      * VectorE runs a custom fused DVE op  min(Src0,Src1) + min-accumulate
        that eats one PSUM stream + one SBUF stream per cycle.
  - Per-query minima are summed on device to [128,1] per core; the final
    scalar is assembled on host: loss = (sum of all mins) * 0.5 / B.
"""

import re
import sys

sys.path.insert(0, "/opt/trn_rl_repo")

import numpy as np

import concourse.bacc as bacc
import concourse.mybir as mybir
import concourse.tile as tile
import concourse.dve_ops as dve_ops
from concourse.bass_interp import get_hw_module
from concourse.bass_utils import run_bass_kernel_spmd
from concourse.dve_ops import DveOp
from concourse.dve_spec import C0, Spec, Src0, Src1, minn

B, N, DIM = 4, 8192, 3
N_CORES = 8
HALF = N // 2              # queries per core per direction
QT = 128                   # queries per tile (partition dim)
NQT = HALF // QT           # query tiles per direction (32)
CT = 512                   # candidates per matmul (one PSUM bank)
SPAN = 1024                # candidates per PSUM span (2 banks)
K = 24                     # augmented contraction dim (bf16 3-way split)
F32 = mybir.dt.float32
BF16 = mybir.dt.bfloat16

# DRAM input layout per core: one [K, 24576] fp32 tensor with columns
#   [rhs_dirA (8192) | rhs_dirB (8192) | lhs_dirA (4096) | lhs_dirB (4096)]
IN_COLS = 2 * N + 2 * HALF


# --- custom DVE op: out = min(in0, in1); accum_out = min(s0, min_k out) ----
def _min2_ref(in0, in1, s0, s1, imm2):
    b = np.minimum(in0, in1).astype(np.float32)
    m = b.reshape(b.shape[0], -1).min(axis=-1, keepdims=True)
    s0 = np.broadcast_to(np.asarray(s0, np.float32), m.shape)
    return b, np.minimum(s0, m).astype(np.float32)


def _register_min2():
    for op in dve_ops.OPS:
        if op.name == "MIN2_ACC_CD":
            return op
    op = DveOp(
        "MIN2_ACC_CD",
        Spec(body=minn(Src0, Src1), accum=minn, accum_init=C0, reference=_min2_ref),
        subdim=False,
        uops_sha={},
    )
    dve_ops.OPS.append(op)
    dve_ops.CUSTOM_DVE_SPECS[op.name] = op.spec
    dve_ops._SUB_OPCODE_FOR_NAME[op.name] = (
        dve_ops._CUSTOM_DVE_ROW_BASE + len(dve_ops.OPS) - 1
    )
    for ver in ("v3", "v4"):
        try:
            op.compile(ver)
        except ValueError as e:
            m = re.search(r'"([0-9a-f]{16})"', str(e))
            op.uops_sha[ver] = m.group(1)
            op.compile(ver)
    return op


MIN2 = _register_min2()


# --- device program ---------------------------------------------------------
def _build_program():
    nc = bacc.Bacc(
        trn_type="TRN2", debug=False, num_devices=N_CORES, enable_asserts=False
    )
    inp = nc.dram_tensor("inp", [K, IN_COLS], BF16, kind="ExternalInput")
    out = nc.dram_tensor("out", [128, 1], F32, kind="ExternalOutput")

    with tile.TileContext(nc) as tc:
        with (
            tc.tile_pool(name="const", bufs=1) as cpool,
            tc.tile_pool(name="psA", bufs=2, space="PSUM") as psA,
            tc.tile_pool(name="psB", bufs=2, space="PSUM") as psB,
            tc.tile_pool(name="stage", bufs=3) as stpool,
            tc.tile_pool(name="scr", bufs=2) as scrpool,
            tc.tile_pool(name="acc", bufs=4) as accpool,
        ):
            data = cpool.tile([K, IN_COLS], BF16)
            nc.sync.dma_start(out=data[:], in_=inp.ap())
            minbuf = cpool.tile([128, 2 * NQT], F32)
            sums = cpool.tile([128, 1], F32)

            data_r = data
            for d in range(2):
                rhs = data_r[:, d * N : (d + 1) * N]
                lhs = data_r[:, 2 * N + d * HALF : 2 * N + (d + 1) * HALF]
                for qt in range(NQT):
                    w = lhs[:, qt * QT : (qt + 1) * QT]
                    acc = accpool.tile([128, 4], F32)
                    for r in range(4):
                        base = r * 2 * SPAN
                        pa = psA.tile([128, SPAN], F32)
                        pb = psB.tile([128, SPAN], F32)
                        for half in range(2):
                            nc.tensor.matmul(
                                out=pa[:, half * CT : (half + 1) * CT],
                                lhsT=w,
                                rhs=rhs[:, base + half * CT : base + (half + 1) * CT],
                                start=True,
                                stop=True,
                            )
                        for half in range(2):
                            nc.tensor.matmul(
                                out=pb[:, half * CT : (half + 1) * CT],
                                lhsT=w,
                                rhs=rhs[
                                    :,
                                    base + SPAN + half * CT : base
                                    + SPAN
                                    + (half + 1) * CT,
                                ],
                                start=True,
                                stop=True,
                            )
                        sb = stpool.tile([128, SPAN], F32)
                        nc.scalar.copy(out=sb[:], in_=pb[:])
                        scr = scrpool.tile([128, SPAN], F32)
                        nc.vector._custom_dve(
                            MIN2,
                            out=scr[:],
                            in0=pa[:],
                            in1=sb[:],
                            s0=1.0e30,
                            accum_out=acc[:, r : r + 1],
                        )
                    col = d * NQT + qt
                    nc.vector.tensor_reduce(
                        out=minbuf[:, col : col + 1],
                        in_=acc[:],
                        axis=mybir.AxisListType.X,
                        op=mybir.AluOpType.min,
                    )
            nc.vector.tensor_reduce(
                out=sums[:],
                in_=minbuf[:],
                axis=mybir.AxisListType.X,
                op=mybir.AluOpType.add,
            )
            nc.sync.dma_start(out=out.ap(), in_=sums[:])

    nc.compile()
    nc.m = get_hw_module(nc.m)
    return nc


_NC = None


def _get_nc():
    global _NC
    if _NC is None:
        _NC = _build_program()
    return _NC


# --- host-side packing ------------------------------------------------------
import ml_dtypes

BF = ml_dtypes.bfloat16


def _bf16_split3(a):
    """Round-to-nearest 3-way bf16 split: a ~= a1 + a2 + a3."""
    a = np.ascontiguousarray(a, np.float64)
    a1 = a.astype(np.float32).astype(BF)
    r = a - a1.astype(np.float64)
    a2 = r.astype(np.float32).astype(BF)
    r = r - a2.astype(np.float64)
    a3 = r.astype(np.float32).astype(BF)
    return a1, a2, a3


def _pack_side(q, c):
    """Build (lhs [K, nq], rhs [K, nc]) for queries q [nq,3], candidates c [nc,3].

    D[i,j] = qq_i + cc_j - 2 q_i.c_j via bf16 products:
      qq ~ qq1+qq2+qq3 (rows 0-2 x ones), cc likewise (rows 3-5),
      q.c ~ q1c1 + q1c2 + q2c1 + q2c2 + q1c3 + q3c1 per dim (rows 6-23).
    """
    nq, ncand = q.shape[0], c.shape[0]
    qq = (q.astype(np.float64) ** 2).sum(-1)
    cc = (c.astype(np.float64) ** 2).sum(-1)
    qq1, qq2, qq3 = _bf16_split3(qq)
    cc1, cc2, cc3 = _bf16_split3(cc)
    q1, q2, q3 = _bf16_split3(q)
    c1, c2, c3 = _bf16_split3(c)

    ones_q = np.ones(nq, BF)
    ones_c = np.ones(ncand, BF)

    lhs = np.empty((K, nq), BF)
    rhs = np.empty((K, ncand), BF)
    lhs[0], lhs[1], lhs[2] = qq1, qq2, qq3
    rhs[0] = rhs[1] = rhs[2] = ones_c
    lhs[3] = lhs[4] = lhs[5] = ones_q
    rhs[3], rhs[4], rhs[5] = cc1, cc2, cc3

    def m2(x):
        return (-2.0 * x.astype(np.float32)).astype(BF)

    for d in range(DIM):
        base = 6 + 6 * d
        lq = [m2(q1[:, d]), m2(q1[:, d]), m2(q2[:, d]),
              m2(q2[:, d]), m2(q1[:, d]), m2(q3[:, d])]
        rc = [c1[:, d], c2[:, d], c1[:, d], c2[:, d], c3[:, d], c1[:, d]]
        for k in range(6):
            lhs[base + k] = lq[k]
            rhs[base + k] = rc[k]
    return lhs, rhs


def _pack_core(x, y, core):
    b, h = core // 2, core % 2
    sl = slice(h * HALF, (h + 1) * HALF)
    lhsA, rhsA = _pack_side(x[b, sl], y[b])
    lhsB, rhsB = _pack_side(y[b, sl], x[b])
    buf = np.empty((K, IN_COLS), BF)
    buf[:, 0:N] = rhsA
    buf[:, N : 2 * N] = rhsB
    buf[:, 2 * N : 2 * N + HALF] = lhsA
    buf[:, 2 * N + HALF :] = lhsB
    return buf


def kernel(gen_points_batch, train_points_dense_batch, _profile=None):
    x = np.ascontiguousarray(gen_points_batch, np.float32)
    y = np.ascontiguousarray(train_points_dense_batch, np.float32)
    assert x.shape == (B, N, DIM) and y.shape == (B, N, DIM)

    in_maps = [{"inp": _pack_core(x, y, c)} for c in range(N_CORES)]
    nc = _get_nc()
    res = run_bass_kernel_spmd(
        nc, in_maps, list(range(N_CORES)), **(_profile or {})
    )
    total = sum(
        res.results[c]["out"].astype(np.float64).sum() for c in range(N_CORES)
    )
    loss = np.float32(total * 0.5 / B)
    if _profile:
        kernel._last_result = res
    return loss


# revision 4
# speedup vs baseline: 1.3275x; 1.3074x over previous
"""Chamfer-distance (CDLoss) kernel for 8x Trainium2 NeuronCores.

Strategy (data-parallel, per spec sharding hint):
  - 8 cores = 4 batches x 2 halves. Core c handles batch b=c//2, query-half
    h=c%2 for BOTH directions of the chamfer distance:
      dir A: queries = x[b, h*4096:(h+1)*4096], candidates = y[b] (all 8192)
      dir B: queries = y[b, h*4096:(h+1)*4096], candidates = x[b]
  - On device, per query-tile of 128 (partition dim), the full distance
    matrix D[i,j] = |q_i|^2 + |c_j|^2 - 2 q_i.c_j is produced by TensorE as
    an augmented matmul (K=13) in float32r (tf32) with an exact hi/lo split
    compensation, 512 candidates per PSUM bank.
  - Min-reduction consumes PSUM via two paths in parallel:
      * ScalarE copies half the banks to SBUF (fp32),
      * VectorE runs a custom fused DVE op  min(Src0,Src1) + min-accumulate
        that eats one PSUM stream + one SBUF stream per cycle.
  - Per-query minima are summed on device to [128,1] per core; the final
    scalar is assembled on host: loss = (sum of all mins) * 0.5 / B.
"""

import re
import sys

sys.path.insert(0, "/opt/trn_rl_repo")

import numpy as np

import concourse.bacc as bacc
import concourse.mybir as mybir
import concourse.tile as tile
import concourse.dve_ops as dve_ops
from concourse.bass_interp import get_hw_module
from concourse.bass_utils import run_bass_kernel_spmd
from concourse.dve_ops import DveOp
from concourse.dve_spec import C0, Spec, Src0, Src1, minn

B, N, DIM = 4, 8192, 3
N_CORES = 8
HALF = N // 2              # queries per core per direction
QT = 128                   # queries per tile (partition dim)
NQT = HALF // QT           # query tiles per direction (32)
CT = 512                   # candidates per matmul (one PSUM bank)
SPAN = 1024                # candidates per PSUM span (2 banks)
K = 24                     # augmented contraction dim (bf16 3-way split)
F32 = mybir.dt.float32
BF16 = mybir.dt.bfloat16

# DRAM input layout per core: one [K, 24576] fp32 tensor with columns
#   [rhs_dirA (8192) | rhs_dirB (8192) | lhs_dirA (4096) | lhs_dirB (4096)]
IN_COLS = 2 * N + 2 * HALF


# --- custom DVE op: out = min(in0, in1); accum_out = min(s0, min_k out) ----
def _min2_ref(in0, in1, s0, s1, imm2):
    b = np.minimum(in0, in1).astype(np.float32)
    m = b.reshape(b.shape[0], -1).min(axis=-1, keepdims=True)
    s0 = np.broadcast_to(np.asarray(s0, np.float32), m.shape)
    return b, np.minimum(s0, m).astype(np.float32)


def _register_min2():
    for op in dve_ops.OPS:
        if op.name == "MIN2_ACC_CD":
            return op
    op = DveOp(
        "MIN2_ACC_CD",
        Spec(body=minn(Src0, Src1), accum=minn, accum_init=C0, reference=_min2_ref),
        subdim=False,
        uops_sha={},
    )
    dve_ops.OPS.append(op)
    dve_ops.CUSTOM_DVE_SPECS[op.name] = op.spec
    dve_ops._SUB_OPCODE_FOR_NAME[op.name] = (
        dve_ops._CUSTOM_DVE_ROW_BASE + len(dve_ops.OPS) - 1
    )
    for ver in ("v3", "v4"):
        try:
            op.compile(ver)
        except ValueError as e:
            m = re.search(r'"([0-9a-f]{16})"', str(e))
            op.uops_sha[ver] = m.group(1)
            op.compile(ver)
    return op


MIN2 = _register_min2()


# --- device program ---------------------------------------------------------
def _build_program():
    nc = bacc.Bacc(
        trn_type="TRN2", debug=False, num_devices=N_CORES, enable_asserts=False
    )
    inp = nc.dram_tensor("inp", [64, IN_COLS], BF16, kind="ExternalInput")
    out = nc.dram_tensor("out", [128, 1], F32, kind="ExternalOutput")

    with tile.TileContext(nc) as tc:
        with (
            tc.tile_pool(name="const", bufs=1) as cpool,
            tc.tile_pool(name="psA", bufs=1, space="PSUM") as psA,
            tc.tile_pool(name="psB", bufs=1, space="PSUM") as psB,
            tc.tile_pool(name="stage", bufs=3) as stpool,
            tc.tile_pool(name="scr", bufs=2) as scrpool,
            tc.tile_pool(name="acc", bufs=4) as accpool,
        ):
            data = cpool.tile([64, IN_COLS], BF16)
            nc.sync.dma_start(out=data[:], in_=inp.ap())
            minbuf = cpool.tile([128, 2 * NQT], F32)
            sums = cpool.tile([128, 1], F32)

            # row-group views: group 0 at partitions 0-23, group 1 at 32-55
            grp = [data[0:K, :], data[32 : 32 + K, :]]
            for d in range(2):
                rhs = [g[:, d * N : (d + 1) * N] for g in grp]
                lhs = [g[:, 2 * N + d * HALF : 2 * N + (d + 1) * HALF] for g in grp]
                for t in range(NQT // 2):
                    qts = (2 * t, 2 * t + 1)
                    ws = [lhs[g][:, qts[g] * QT : (qts[g] + 1) * QT] for g in range(2)]
                    accs = [accpool.tile([128, 4], F32, name=f"acc{g}") for g in range(2)]
                    for r in range(4):
                        base = r * 2 * SPAN
                        pas = [psA.tile([128, SPAN], F32, name=f"pa{g}") for g in range(2)]
                        pbs = [psB.tile([128, SPAN], F32, name=f"pb{g}") for g in range(2)]
                        for half in range(2):
                            for g in range(2):
                                nc.tensor.matmul(
                                    out=pas[g][:, half * CT : (half + 1) * CT],
                                    lhsT=ws[g],
                                    rhs=rhs[g][:, base + half * CT : base + (half + 1) * CT],
                                    start=True,
                                    stop=True,
                                )
                        for half in range(2):
                            for g in range(2):
                                nc.tensor.matmul(
                                    out=pbs[g][:, half * CT : (half + 1) * CT],
                                    lhsT=ws[g],
                                    rhs=rhs[g][
                                        :,
                                        base + SPAN + half * CT : base
                                        + SPAN
                                        + (half + 1) * CT,
                                    ],
                                    start=True,
                                    stop=True,
                                )
                        for g in range(2):
                            sb = stpool.tile([128, SPAN], F32, name=f"sb{g}")
                            nc.scalar.copy(out=sb[:], in_=pbs[g][:])
                            scr = scrpool.tile([128, SPAN], F32, name=f"scr{g}")
                            nc.vector._custom_dve(
                                MIN2,
                                out=scr[:],
                                in0=pas[g][:],
                                in1=sb[:],
                                s0=1.0e30,
                                accum_out=accs[g][:, r : r + 1],
                            )
                    for g in range(2):
                        col = d * NQT + qts[g]
                        nc.vector.tensor_reduce(
                            out=minbuf[:, col : col + 1],
                            in_=accs[g][:],
                            axis=mybir.AxisListType.X,
                            op=mybir.AluOpType.min,
                        )
            nc.vector.tensor_reduce(
                out=sums[:],
                in_=minbuf[:],
                axis=mybir.AxisListType.X,
                op=mybir.AluOpType.add,
            )
            nc.sync.dma_start(out=out.ap(), in_=sums[:])

    nc.compile()
    nc.m = get_hw_module(nc.m)
    return nc


_NC = None


def _get_nc():
    global _NC
    if _NC is None:
        _NC = _build_program()
    return _NC


# --- host-side packing ------------------------------------------------------
import ml_dtypes

BF = ml_dtypes.bfloat16


def _bf16_split3(a):
    """Round-to-nearest 3-way bf16 split: a ~= a1 + a2 + a3."""
    a = np.ascontiguousarray(a, np.float64)
    a1 = a.astype(np.float32).astype(BF)
    r = a - a1.astype(np.float64)
    a2 = r.astype(np.float32).astype(BF)
    r = r - a2.astype(np.float64)
    a3 = r.astype(np.float32).astype(BF)
    return a1, a2, a3


def _pack_side(q, c):
    """Build (lhs [K, nq], rhs [K, nc]) for queries q [nq,3], candidates c [nc,3].

    D[i,j] = qq_i + cc_j - 2 q_i.c_j via bf16 products:
      qq ~ qq1+qq2+qq3 (rows 0-2 x ones), cc likewise (rows 3-5),
      q.c ~ q1c1 + q1c2 + q2c1 + q2c2 + q1c3 + q3c1 per dim (rows 6-23).
    """
    nq, ncand = q.shape[0], c.shape[0]
    qq = (q.astype(np.float64) ** 2).sum(-1)
    cc = (c.astype(np.float64) ** 2).sum(-1)
    qq1, qq2, qq3 = _bf16_split3(qq)
    cc1, cc2, cc3 = _bf16_split3(cc)
    q1, q2, q3 = _bf16_split3(q)
    c1, c2, c3 = _bf16_split3(c)

    ones_q = np.ones(nq, BF)
    ones_c = np.ones(ncand, BF)

    lhs = np.empty((K, nq), BF)
    rhs = np.empty((K, ncand), BF)
    lhs[0], lhs[1], lhs[2] = qq1, qq2, qq3
    rhs[0] = rhs[1] = rhs[2] = ones_c
    lhs[3] = lhs[4] = lhs[5] = ones_q
    rhs[3], rhs[4], rhs[5] = cc1, cc2, cc3

    def m2(x):
        return (-2.0 * x.astype(np.float32)).astype(BF)

    for d in range(DIM):
        base = 6 + 6 * d
        lq = [m2(q1[:, d]), m2(q1[:, d]), m2(q2[:, d]),
              m2(q2[:, d]), m2(q1[:, d]), m2(q3[:, d])]
        rc = [c1[:, d], c2[:, d], c1[:, d], c2[:, d], c3[:, d], c1[:, d]]
        for k in range(6):
            lhs[base + k] = lq[k]
            rhs[base + k] = rc[k]
    return lhs, rhs


def _pack_core(x, y, core):
    b, h = core // 2, core % 2
    sl = slice(h * HALF, (h + 1) * HALF)
    lhsA, rhsA = _pack_side(x[b, sl], y[b])
    lhsB, rhsB = _pack_side(y[b, sl], x[b])
    buf = np.zeros((64, IN_COLS), BF)
    buf[0:K, 0:N] = rhsA
    buf[0:K, N : 2 * N] = rhsB
    buf[0:K, 2 * N : 2 * N + HALF] = lhsA
    buf[0:K, 2 * N + HALF :] = lhsB
    buf[32 : 32 + K, :] = buf[0:K, :]
    return buf


def kernel(gen_points_batch, train_points_dense_batch, _profile=None):
    x = np.ascontiguousarray(gen_points_batch, np.float32)
    y = np.ascontiguousarray(train_points_dense_batch, np.float32)
    assert x.shape == (B, N, DIM) and y.shape == (B, N, DIM)

    in_maps = [{"inp": _pack_core(x, y, c)} for c in range(N_CORES)]
    nc = _get_nc()
    res = run_bass_kernel_spmd(
        nc, in_maps, list(range(N_CORES)), **(_profile or {})
    )
    total = sum(
        res.results[c]["out"].astype(np.float64).sum() for c in range(N_CORES)
    )
    loss = np.float32(total * 0.5 / B)
    if _profile:
        kernel._last_result = res
    return loss


# revision 5
# speedup vs baseline: 1.3385x; 1.0083x over previous
"""Chamfer-distance (CDLoss) kernel for 8x Trainium2 NeuronCores.

Strategy (data-parallel, per spec sharding hint):
  - 8 cores = 4 batches x 2 halves. Core c handles batch b=c//2, query-half
    h=c%2 for BOTH directions of the chamfer distance:
      dir A: queries = x[b, h*4096:(h+1)*4096], candidates = y[b] (all 8192)
      dir B: queries = y[b, h*4096:(h+1)*4096], candidates = x[b]
  - On device, per query-tile of 128 (partition dim), the full distance
    matrix D[i,j] = |q_i|^2 + |c_j|^2 - 2 q_i.c_j is produced by TensorE as
    an augmented matmul (K=13) in float32r (tf32) with an exact hi/lo split
    compensation, 512 candidates per PSUM bank.
  - Min-reduction consumes PSUM via two paths in parallel:
      * ScalarE copies half the banks to SBUF (fp32),
      * VectorE runs a custom fused DVE op  min(Src0,Src1) + min-accumulate
        that eats one PSUM stream + one SBUF stream per cycle.
  - Per-query minima are summed on device to [128,1] per core; the final
    scalar is assembled on host: loss = (sum of all mins) * 0.5 / B.
"""

import re
import sys

sys.path.insert(0, "/opt/trn_rl_repo")

import numpy as np

import concourse.bacc as bacc
import concourse.mybir as mybir
import concourse.tile as tile
import concourse.dve_ops as dve_ops
from concourse.bass_interp import get_hw_module
from concourse.bass_utils import run_bass_kernel_spmd
from concourse.dve_ops import DveOp
from concourse.dve_spec import C0, Spec, Src0, Src1, minn

B, N, DIM = 4, 8192, 3
N_CORES = 8
HALF = N // 2              # queries per core per direction
QT = 128                   # queries per tile (partition dim)
NQT = HALF // QT           # query tiles per direction (32)
CT = 512                   # candidates per matmul (one PSUM bank)
SPAN = 1024                # candidates per PSUM span (2 banks)
K = 24                     # augmented contraction dim (bf16 3-way split)
F32 = mybir.dt.float32
BF16 = mybir.dt.bfloat16

# DRAM input layout per core: one [K, 24576] fp32 tensor with columns
#   [rhs_dirA (8192) | rhs_dirB (8192) | lhs_dirA (4096) | lhs_dirB (4096)]
IN_COLS = 2 * N + 2 * HALF


# --- custom DVE op: out = min(in0, in1); accum_out = min(s0, min_k out) ----
def _min2_ref(in0, in1, s0, s1, imm2):
    b = np.minimum(in0, in1).astype(np.float32)
    m = b.reshape(b.shape[0], -1).min(axis=-1, keepdims=True)
    s0 = np.broadcast_to(np.asarray(s0, np.float32), m.shape)
    return b, np.minimum(s0, m).astype(np.float32)


def _register_min2():
    for op in dve_ops.OPS:
        if op.name == "MIN2_ACC_CD":
            return op
    op = DveOp(
        "MIN2_ACC_CD",
        Spec(body=minn(Src0, Src1), accum=minn, accum_init=C0, reference=_min2_ref),
        subdim=False,
        uops_sha={},
    )
    dve_ops.OPS.append(op)
    dve_ops.CUSTOM_DVE_SPECS[op.name] = op.spec
    dve_ops._SUB_OPCODE_FOR_NAME[op.name] = (
        dve_ops._CUSTOM_DVE_ROW_BASE + len(dve_ops.OPS) - 1
    )
    for ver in ("v3", "v4"):
        try:
            op.compile(ver)
        except ValueError as e:
            m = re.search(r'"([0-9a-f]{16})"', str(e))
            op.uops_sha[ver] = m.group(1)
            op.compile(ver)
    return op


MIN2 = _register_min2()


# --- device program ---------------------------------------------------------
def _build_program():
    nc = bacc.Bacc(
        trn_type="TRN2", debug=False, num_devices=N_CORES, enable_asserts=False
    )
    inp = nc.dram_tensor("inp", [64, IN_COLS], BF16, kind="ExternalInput")
    out = nc.dram_tensor("out", [128, 1], F32, kind="ExternalOutput")

    with tile.TileContext(nc) as tc:
        with (
            tc.tile_pool(name="const", bufs=1) as cpool,
            tc.tile_pool(name="psA", bufs=1, space="PSUM") as psA,
            tc.tile_pool(name="psB", bufs=1, space="PSUM") as psB,
            tc.tile_pool(name="stage", bufs=4) as stpool,
            tc.tile_pool(name="scr", bufs=4) as scrpool,
            tc.tile_pool(name="acc", bufs=4) as accpool,
        ):
            data = cpool.tile([64, IN_COLS], BF16)
            # split input DMA so d=0 compute starts before d=1 data lands
            nc.sync.dma_start(out=data[:, 0:N], in_=inp.ap()[:, 0:N])
            nc.sync.dma_start(
                out=data[:, 2 * N : IN_COLS], in_=inp.ap()[:, 2 * N : IN_COLS]
            )
            nc.sync.dma_start(out=data[:, N : 2 * N], in_=inp.ap()[:, N : 2 * N])
            minbuf = cpool.tile([128, 2 * NQT], F32)
            sums = cpool.tile([128, 1], F32)

            # row-group views: group 0 at partitions 0-23, group 1 at 32-55
            grp = [data[0:K, :], data[32 : 32 + K, :]]
            for d in range(2):
                rhs = [g[:, d * N : (d + 1) * N] for g in grp]
                lhs = [g[:, 2 * N + d * HALF : 2 * N + (d + 1) * HALF] for g in grp]
                for t in range(NQT // 2):
                    qts = (2 * t, 2 * t + 1)
                    ws = [lhs[g][:, qts[g] * QT : (qts[g] + 1) * QT] for g in range(2)]
                    accs = [accpool.tile([128, 4], F32, name=f"acc{g}") for g in range(2)]
                    for r in range(4):
                        base = r * 2 * SPAN
                        pas = [psA.tile([128, SPAN], F32, name=f"pa{g}") for g in range(2)]
                        pbs = [psB.tile([128, SPAN], F32, name=f"pb{g}") for g in range(2)]
                        for half in range(2):
                            for g in range(2):
                                nc.tensor.matmul(
                                    out=pbs[g][:, half * CT : (half + 1) * CT],
                                    lhsT=ws[g],
                                    rhs=rhs[g][
                                        :,
                                        base + SPAN + half * CT : base
                                        + SPAN
                                        + (half + 1) * CT,
                                    ],
                                    start=True,
                                    stop=True,
                                )
                        sbs = [stpool.tile([128, SPAN], F32, name=f"sb{g}") for g in range(2)]
                        for g in range(2):
                            nc.scalar.copy(out=sbs[g][:], in_=pbs[g][:])
                        for half in range(2):
                            for g in range(2):
                                nc.tensor.matmul(
                                    out=pas[g][:, half * CT : (half + 1) * CT],
                                    lhsT=ws[g],
                                    rhs=rhs[g][:, base + half * CT : base + (half + 1) * CT],
                                    start=True,
                                    stop=True,
                                )
                        for g in range(2):
                            scr = scrpool.tile([128, SPAN], F32, name=f"scr{g}")
                            nc.vector._custom_dve(
                                MIN2,
                                out=scr[:],
                                in0=pas[g][:],
                                in1=sbs[g][:],
                                s0=1.0e30,
                                accum_out=accs[g][:, r : r + 1],
                            )
                    for g in range(2):
                        col = d * NQT + qts[g]
                        nc.vector.tensor_reduce(
                            out=minbuf[:, col : col + 1],
                            in_=accs[g][:],
                            axis=mybir.AxisListType.X,
                            op=mybir.AluOpType.min,
                        )
            nc.vector.tensor_reduce(
                out=sums[:],
                in_=minbuf[:],
                axis=mybir.AxisListType.X,
                op=mybir.AluOpType.add,
            )
            nc.sync.dma_start(out=out.ap(), in_=sums[:])

    nc.compile()
    nc.m = get_hw_module(nc.m)
    return nc


_NC = None


def _get_nc():
    global _NC
    if _NC is None:
        _NC = _build_program()
    return _NC


# --- host-side packing ------------------------------------------------------
import ml_dtypes

BF = ml_dtypes.bfloat16


def _bf16_split3(a):
    """Round-to-nearest 3-way bf16 split: a ~= a1 + a2 + a3."""
    a = np.ascontiguousarray(a, np.float64)
    a1 = a.astype(np.float32).astype(BF)
    r = a - a1.astype(np.float64)
    a2 = r.astype(np.float32).astype(BF)
    r = r - a2.astype(np.float64)
    a3 = r.astype(np.float32).astype(BF)
    return a1, a2, a3


def _pack_side(q, c):
    """Build (lhs [K, nq], rhs [K, nc]) for queries q [nq,3], candidates c [nc,3].

    D[i,j] = qq_i + cc_j - 2 q_i.c_j via bf16 products:
      qq ~ qq1+qq2+qq3 (rows 0-2 x ones), cc likewise (rows 3-5),
      q.c ~ q1c1 + q1c2 + q2c1 + q2c2 + q1c3 + q3c1 per dim (rows 6-23).
    """
    nq, ncand = q.shape[0], c.shape[0]
    qq = (q.astype(np.float64) ** 2).sum(-1)
    cc = (c.astype(np.float64) ** 2).sum(-1)
    qq1, qq2, qq3 = _bf16_split3(qq)
    cc1, cc2, cc3 = _bf16_split3(cc)
    q1, q2, q3 = _bf16_split3(q)
    c1, c2, c3 = _bf16_split3(c)

    ones_q = np.ones(nq, BF)
    ones_c = np.ones(ncand, BF)

    lhs = np.empty((K, nq), BF)
    rhs = np.empty((K, ncand), BF)
    lhs[0], lhs[1], lhs[2] = qq1, qq2, qq3
    rhs[0] = rhs[1] = rhs[2] = ones_c
    lhs[3] = lhs[4] = lhs[5] = ones_q
    rhs[3], rhs[4], rhs[5] = cc1, cc2, cc3

    def m2(x):
        return (-2.0 * x.astype(np.float32)).astype(BF)

    for d in range(DIM):
        base = 6 + 6 * d
        lq = [m2(q1[:, d]), m2(q1[:, d]), m2(q2[:, d]),
              m2(q2[:, d]), m2(q1[:, d]), m2(q3[:, d])]
        rc = [c1[:, d], c2[:, d], c1[:, d], c2[:, d], c3[:, d], c1[:, d]]
        for k in range(6):
            lhs[base + k] = lq[k]
            rhs[base + k] = rc[k]
    return lhs, rhs


def _pack_core(x, y, core):
    b, h = core // 2, core % 2
    sl = slice(h * HALF, (h + 1) * HALF)
    lhsA, rhsA = _pack_side(x[b, sl], y[b])
    lhsB, rhsB = _pack_side(y[b, sl], x[b])
    buf = np.zeros((64, IN_COLS), BF)
    buf[0:K, 0:N] = rhsA
    buf[0:K, N : 2 * N] = rhsB
    buf[0:K, 2 * N : 2 * N + HALF] = lhsA
    buf[0:K, 2 * N + HALF :] = lhsB
    buf[32 : 32 + K, :] = buf[0:K, :]
    return buf


def kernel(gen_points_batch, train_points_dense_batch, _profile=None):
    x = np.ascontiguousarray(gen_points_batch, np.float32)
    y = np.ascontiguousarray(train_points_dense_batch, np.float32)
    assert x.shape == (B, N, DIM) and y.shape == (B, N, DIM)

    in_maps = [{"inp": _pack_core(x, y, c)} for c in range(N_CORES)]
    nc = _get_nc()
    res = run_bass_kernel_spmd(
        nc, in_maps, list(range(N_CORES)), **(_profile or {})
    )
    total = sum(
        res.results[c]["out"].astype(np.float64).sum() for c in range(N_CORES)
    )
    loss = np.float32(total * 0.5 / B)
    if _profile:
        kernel._last_result = res
    return loss
